# revision 1
# baseline (speedup 1.0000x reference)
"""Causal multi-head attention on 8 TRN2 NeuronCores.

Problem: x[4,2048,1024], w_attn[1024,3072], w_proj[1024,1024],
16 heads x 64 dim, causal softmax(QK^T/8)V then output projection.

Sharding: 4-way batch x 2-way head-half. Core c handles batch c//2 and
heads (c%2)*8 .. (c%2)*8+8. Each core computes a partial y^T (its head
half's contribution to the output projection); the host sums the two
partials per batch and transposes.

Per-core layout strategy (all matmuls fp32r, 1 cycle/row at N>=256):
 - host feeds x^T [1024, 2048] (c_in-major)
 - QKV projection: Q^T,K^T computed feature-major [512, T]; V computed
   token-major [T, 512] (so no on-device transposes anywhere)
 - attention computed transposed: S^T[k,q] = (K^T).T-slices @ Q^T with
   K=64 contraction run as PE 64x128 row-tile pairs (two heads at
   partition bases 0/64 execute concurrently on T0/T8)
 - P = exp(S^T/8) on ACT straight out of PSUM (bf16 output); causal
   handling: sub-diagonal chunks skipped, diagonal chunks computed from
   their causal offset, the 128-wide diagonal strip masked with a
   triangular multiply, left remainder zero-filled
 - PV: O^T[d,q] accumulated over key tiles in bf16 with stationary
   [V_h | 1 | 0-pad] padded to M=128 (keeps fast weight load); PSUM
   row 64 carries the softmax denominators for free
 - software pipeline: S^T/exp of unit i interleaved with PV of unit
   i-1 so the in-order PE stream always has matmul work while ACT
   chews through the exps
 - normalize via DVE reciprocal + stream_shuffle partition broadcast
 - projection: y^T partial = w_proj_slice.T-rows @ O^T
"""

import dataclasses
import numpy as np
from contextlib import ExitStack

import concourse.bass as bass
import concourse.tile as tile
from concourse import bacc, mybir
from concourse.bass_utils import run_bass_kernel_spmd

f32 = mybir.dt.float32
f32r = mybir.dt.float32r
bf16 = mybir.dt.bfloat16
EXP = mybir.ActivationFunctionType.Exp

B, T, C = 4, 2048, 1024
N_HEAD, HD = 16, 64
HPC = 8            # heads per core
FS = HPC * HD      # 512: per-core feature slice for each of q/k/v
NPAIR = HPC // 2   # 4 head pairs
SCALE = 1.0 / 8.0  # 1/sqrt(64)
N_CORES = 8


def build_nc(tpc=T, loop_n=1, dyn_loop=0, stages='ABC'):
    """Build the single-core Bass program (SPMD: same program all cores).

    loop_n > 1 unrolls the whole body N times inside one NEFF (timing
    instrument: device time per body = (T(N) - T(1)) / (N - 1)).
    """
    nck = C // 128          # 8 c_in tiles
    nkt = tpc // 128        # key tiles
    nqc = tpc // 512        # query chunks (512 wide)
    nmt = C // 128          # 8 output-channel tiles

    nc = bacc.Bacc("TRN2", target_bir_lowering=False)
    xt = nc.dram_tensor("xt", [C, tpc], bf16, kind="ExternalInput")
    wq = nc.dram_tensor("wq", [C, FS], bf16, kind="ExternalInput")
    wk = nc.dram_tensor("wk", [C, FS], bf16, kind="ExternalInput")
    wv = nc.dram_tensor("wv", [C, FS], bf16, kind="ExternalInput")
    wp = nc.dram_tensor("wp", [FS, C], bf16, kind="ExternalInput")
    mk = nc.dram_tensor("mk", [128, 128], bf16, kind="ExternalInput")
    yt = nc.dram_tensor("yt", [C, tpc], f32, kind="ExternalOutput")

    with tile.TileContext(nc) as tc, ExitStack() as _dl:
     if dyn_loop:
        _dl.enter_context(tc.For_i(0, dyn_loop, 1))
     for _rep in range(loop_n):
      with ExitStack() as stk:
            # tensors that live across stages
            persist = stk.enter_context(tc.tile_pool(name="persist", bufs=1))
            qT = [persist.tile([128, tpc], bf16, tag=f"qT{p}", name=f"qT{p}") for p in range(NPAIR)]
            kT = [persist.tile([128, tpc], bf16, tag=f"kT{p}", name=f"kT{p}") for p in range(NPAIR)]
            # vhat[kt]: [128 keys, 8 heads, 64 dims + ones column]
            vhat = [persist.tile([128, HPC, 128], bf16, tag=f"vh{t}", name=f"vh{t}")
                    for t in range(nkt)]
            mkt = persist.tile([128, 128], bf16, tag="mk")
            nc.sync.dma_start(out=mkt, in_=mk[:, :])
            ones_f = persist.tile([128, HPC], f32, tag="ones")
            nc.vector.memset(ones_f[:, :], 1.0)
            # persistent f32r zeros for left-filling diagonal P chunks
            zeros_f = persist.tile([128, 504], f32, tag="zf")
            nc.vector.memset(zeros_f[:, :], 0.0)
            zeros_r = persist.tile([128, 504], bf16, tag="zr")
            nc.vector.tensor_copy(zeros_r[:, :], zeros_f[:, :])
            # persistent shuffle inputs (2, alternating): only rows 0/32 rewritten
            bcis = []
            for bi in range(2):
                b_ = persist.tile([64, 512], f32, tag=f"bci{bi}", name=f"bci{bi}")
                nc.vector.memset(b_[:, :], 0.0)
                bcis.append(b_)

            # ---------------- Stage A: Q/K projection ----------------
            xa = stk.enter_context(tc.tile_pool(name="xa", bufs=1))
            wb = stk.enter_context(tc.tile_pool(name="wb", bufs=1))
            with tc.tile_pool(name="wa", bufs=3) as wa, \
                 tc.tile_pool(name="psa", bufs=4, space="PSUM") as psa:
                xts = []
                for i in range(nck):
                    x_i = xa.tile([128, tpc], bf16, tag=f"x{i}")
                    nc.sync.dma_start(out=x_i, in_=xt[i * 128:(i + 1) * 128, :])
                    xts.append(x_i)

                # Q^T / K^T feature-major: out[feat, tok]
                for dst, wsrc in ((qT, wq), (kT, wk)):
                    for m in range(NPAIR):
                        wt = wa.tile([128, nck, 128], bf16, tag="wqk")
                        nc.sync.dma_start(
                            out=wt,
                            in_=wsrc.rearrange("(a p) f -> p a f", p=128)[
                                :, :, m * 128:(m + 1) * 128])
                        for n in range(tpc // 512):
                            ps = psa.tile([128, 512], f32, tag="ps")
                            for k in range(nck):
                                nc.tensor.matmul(
                                    ps[:, :], wt[:, k, :],
                                    xts[k][:, n * 512:(n + 1) * 512],
                                    start=(k == 0), stop=(k == nck - 1))
                            nc.scalar.activation(
                                dst[m][:, n * 512:(n + 1) * 512], ps[:, :],
                                mybir.ActivationFunctionType.Copy)

                # V weights loaded here; V compute is interleaved into B
                wvt = wb.tile([128, nck, FS], bf16, tag="wv")
                nc.sync.dma_start(out=wvt, in_=wv.rearrange("(a p) f -> p a f", p=128))

            # ---------------- Stages B+C ----------------
            otp = stk.enter_context(tc.tile_pool(name="ot", bufs=1))
            oT = [otp.tile([128, tpc], bf16, tag=f"oT{p}", name=f"oT{p}") for p in range(NPAIR)]

            with tc.tile_pool(name="pp", bufs=46) as pp, \
                 tc.tile_pool(name="rp", bufs=4) as rp, \
                 tc.tile_pool(name="wc", bufs=1) as wc, \
                 tc.tile_pool(name="ev", bufs=2) as ev, \
                 tc.tile_pool(name="psS", bufs=5, space="PSUM") as psS, \
                 tc.tile_pool(name="psO", bufs=3, space="PSUM") as psO:
                # qc-major unit order so each output-projection chunk can be
                # emitted as soon as all four pairs finish that qc
                units = ([(p, qc) for qc in range(nqc) for p in range(NPAIR)]
                         if 'B' in stages else [])
                wpts = []
                if 'C' in stages and units:
                    for m in range(nmt):
                        wpt = wc.tile([128, NPAIR, 128], bf16, tag=f"wp{m}",
                                      name=f"wp{m}")
                        nc.sync.dma_start(
                            out=wpt,
                            in_=wp.rearrange("(a p) f -> p a f", p=128)[
                                :, :, m * 128:(m + 1) * 128])
                        wpts.append(wpt)

                def emit_c_chunk(n):
                    for m in range(nmt):
                        ps = psO.tile([128, 512], f32, tag="o", name="cps")
                        for j in range(NPAIR):
                            nc.tensor.matmul(
                                ps[:, :], wpts[m][:, j, :],
                                oT[j][:, n * 512:(n + 1) * 512],
                                start=(j == 0), stop=(j == NPAIR - 1))
                        sb = ev.tile([128, 512], f32, tag="sb", name="sb")
                        nc.scalar.activation(sb[:, :], ps[:, :], mybir.ActivationFunctionType.Copy)
                        nc.sync.dma_start(
                            out=yt[m * 128:(m + 1) * 128,
                                   n * 512:(n + 1) * 512],
                            in_=sb)

                vstate = {"t": 0}

                def emit_v_unit():
                    t = vstate["t"]
                    if t >= nkt:
                        return
                    vstate["t"] += 1
                    ps = psS.tile([128, FS], f32, tag="s", name="vps")
                    for k in range(nck):
                        nc.tensor.matmul(
                            ps[:, :], xts[k][:, t * 128:(t + 1) * 128],
                            wvt[:, k, :],
                            start=(k == 0), stop=(k == nck - 1))
                    nc.scalar.activation(
                        vhat[t][:, :, 0:HD],
                        ps[:, :].rearrange("p (h d) -> p h d", h=HPC),
                        mybir.ActivationFunctionType.Copy)
                    nc.vector.tensor_copy(vhat[t][:, :, HD], ones_f[:, :])
                    nc.vector.tensor_copy(
                        vhat[t][:, :, HD + 1:128],
                        zeros_r[:, :].rearrange("p (a b) -> p a b", b=63))

                def emit_s_step(st):
                    """Emit one S^T + exp + mask step; returns False when done."""
                    p, qc, kts, i = st["p"], st["qc"], st["kts"], st["i"]
                    if i >= len(kts):
                        return False
                    kt = kts[i]
                    ksl = slice(kt * 128, (kt + 1) * 128)
                    diag = (kt // 4 == qc)
                    off = 128 * (kt % 4) if diag else 0
                    qs2 = slice(qc * 512 + off, (qc + 1) * 512)
                    prs = []
                    for par in range(2):   # head parity: partitions 0/64
                        row = slice(64 * par, 64 * par + 64)
                        ps = psS.tile([128, 512], f32, tag="s", name="s")
                        nc.tensor.matmul(
                            ps[:, off:512], kT[p][row, ksl],
                            qT[p][row, qs2], start=True, stop=True)
                        pr = pp.tile([128, 512], bf16, tag="P", name="P")
                        if off:
                            nc.vector.tensor_copy(pr[:, 0:off],
                                                  zeros_r[:, 0:off])
                        nc.scalar.activation(pr[:, off:512], ps[:, off:512],
                                             EXP, scale=SCALE)
                        if diag:  # mask the 128-wide diagonal strip
                            nc.vector.tensor_mul(
                                pr[:, off:off + 128],
                                pr[:, off:off + 128], mkt[:, :])
                        prs.append(pr)
                    st["ptiles"].append(prs)
                    st["i"] += 1
                    return True

                def emit_pv_step(st):
                    """Emit one PV accumulation step; returns False when done."""
                    p, kts, j = st["p"], st["kts"], st["j"]
                    if j >= len(kts):
                        return False
                    kt = kts[j]
                    for par in range(2):
                        nc.tensor.matmul(
                            st["po"][par][:, :],
                            vhat[kt][:, 2 * p + par, :],
                            st["ptiles"][j][par][:, :],
                            start=(kt == 0), stop=(kt == kts[-1]))
                    st["j"] += 1
                    return True

                def emit_norm(st):
                    p, qc = st["p"], st["qc"]
                    qsl = slice(qc * 512, (qc + 1) * 512)
                    for par in range(2):
                        po = st["po"][par]
                        bci = bcis[par]
                        rden = rp.tile([1, 512], f32, tag="rden", name="rden")
                        nc.vector.reciprocal(rden[:, :], po[HD:HD + 1, :])
                        nc.vector.tensor_copy(bci[0:1, :], rden[:, :])
                        nc.vector.tensor_copy(bci[32:33, :], rden[:, :])
                        bc = rp.tile([64, 512], f32, tag="bc", name="bc")
                        nc.vector.stream_shuffle(bc[:, :], bci[:, :], [0] * 32)
                        # evacuate O to SBUF via ACT (fast psum path), then
                        # the divide-by-den multiply runs all-SBUF on DVE
                        ob = rp.tile([64, 512], bf16, tag="ob", name="ob")
                        nc.scalar.activation(ob[:, :], po[0:HD, :],
                                             mybir.ActivationFunctionType.Copy)
                        nc.vector.tensor_mul(
                            oT[p][64 * par:64 * par + 64, qsl],
                            ob[:, :], bc[:, :])

                def new_state(p, qc):
                    kts = list(range(min(nkt, 4 * (qc + 1))))
                    return {"p": p, "qc": qc, "kts": kts, "i": 0, "j": 0,
                            "ptiles": [],
                            "po": [psO.tile([128, 512], f32, tag="o",
                                            name="po") for _ in range(2)]}

                # software pipeline: S-phase of unit u interleaved with
                # PV-phase of unit u-1 so PE always has matmul work while
                # ACT chews through u's exps. After the last pair of a qc
                # is normalized, that output-projection chunk is emitted —
                # its PE work overlaps the next qc's ACT-bound S-phase.
                def retire(st):
                    while emit_pv_step(st):
                        pass
                    emit_norm(st)
                    if 'C' in stages and st["p"] == NPAIR - 1:
                        emit_c_chunk(st["qc"])

                if units:
                    for _ in range(4):   # PV of the first unit needs vhat[0..3]
                        emit_v_unit()
                prev = None
                for (p, qc) in units:
                    emit_v_unit()        # one V tile per unit until done
                    cur = new_state(p, qc)
                    more_s = True
                    while more_s:
                        more_s = emit_s_step(cur)
                        if prev is not None:
                            emit_pv_step(prev)
                    if prev is not None:
                        retire(prev)
                    prev = cur
                if prev is not None:
                    retire(prev)
    nc.compile()
    return nc


def _make_masks():
    import ml_dtypes
    k = np.arange(128)[:, None]
    q = np.arange(128)[None, :]
    return (q >= k).astype(ml_dtypes.bfloat16)


_NC_CACHE = {}


def _get_nc(tpc=T):
    if tpc not in _NC_CACHE:
        _NC_CACHE[tpc] = build_nc(tpc)
    return _NC_CACHE[tpc]


def make_in_maps(x, w_attn, w_proj):
    import ml_dtypes
    bf = ml_dtypes.bfloat16
    masks = _make_masks()
    in_maps = []
    for core in range(N_CORES):
        b, hh = core // 2, core % 2
        s = slice(hh * FS, (hh + 1) * FS)
        in_maps.append({
            "xt": np.ascontiguousarray(np.asarray(x[b]).T).astype(bf),
            "wq": np.ascontiguousarray(w_attn[:, s]).astype(bf),
            "wk": np.ascontiguousarray(w_attn[:, C:][:, s]).astype(bf),
            "wv": np.ascontiguousarray(w_attn[:, 2 * C:][:, s]).astype(bf),
            "wp": np.ascontiguousarray(w_proj[hh * FS:(hh + 1) * FS, :]).astype(bf),
            "mk": masks,
        })
    return in_maps


def kernel(x, w_attn, w_proj):
    nc = _get_nc(T)
    in_maps = make_in_maps(x, w_attn, w_proj)
    res = run_bass_kernel_spmd(nc, in_maps, list(range(N_CORES)))
    y = np.empty((B, T, C), np.float32)
    for b in range(B):
        yt = res.results[2 * b]["yt"] + res.results[2 * b + 1]["yt"]
        y[b] = yt.T
    return y

